# revision 25
# baseline (speedup 1.0000x reference)
"""Trainium2 Bass kernel for GQA attention (RoPE + causal) with output projection.

Strategy: DP(batch=4) x TP(2) over 8 NeuronCores. Core c handles batch c//2
and head-half t=c%2: 8 q-heads {8t..8t+7}, 2 kv-heads {2t, 2t+1}. Everything
is bf16 (PE runs bf16 at 1 cycle/row even for 128-wide moving operands, and
it halves DMA + SBUF), accumulation in fp32 PSUM. The score matmuls pack the
4 q-heads of a GQA group into one 512-wide moving operand against the shared
stationary K tile. Softmax row-sums accumulate on the Pool engine; the
cross-partition sum + broadcast run on the PE via ones-matmuls; the
reciprocal runs on the (broadcast) [128,512] tile so the DVE divides all
partitions in parallel. Emission is paced so the PE queue never stalls
(the hardware ramps the PE clock 1.2->2.4 GHz only under continuous load).

Host sums the 2 TP partials per batch (the all-reduce of the TP layout).
"""

import math
from contextlib import ExitStack
from dataclasses import dataclass

import numpy as np
from ml_dtypes import bfloat16

import concourse.bass as bass
import concourse.tile as tile
from concourse import bacc, mybir
from concourse.bass_utils import run_bass_kernel_spmd

F32 = mybir.dt.float32
F32R = mybir.dt.float32r
BF16 = mybir.dt.bfloat16
AF = mybir.ActivationFunctionType
MUL = mybir.AluOpType.mult
ADD = mybir.AluOpType.add


@dataclass(frozen=True)
class Cfg:
    B: int = 4       # batch (DP over 4)
    S: int = 2048    # sequence length
    D: int = 2048    # model dim
    HQC: int = 8     # q-heads per core
    G: int = 2       # kv-heads (GQA groups) per core
    HD: int = 128    # head dim

    @property
    def DT(self):
        return self.D // 128     # 16 d-tiles

    @property
    def QT(self):
        return self.S // 128     # 16 q/k tiles

    @property
    def NOUT(self):
        return self.HQC + 2 * self.G   # 12 projection outputs per core


def r(ap):
    return ap.bitcast(F32R)


def build_program(cfg: Cfg, dbg: bool = False):
    c = cfg
    nc = bacc.Bacc("TRN2", target_bir_lowering=False, debug=False)

    xt_d = nc.dram_tensor("xt", [c.D, c.S], BF16, kind="ExternalInput")
    wqt_d = nc.dram_tensor("wqt", [c.D, c.HQC * c.HD], BF16, kind="ExternalInput")
    wkt_d = nc.dram_tensor("wkt", [c.D, c.G * c.HD], BF16, kind="ExternalInput")
    wvt_d = nc.dram_tensor("wvt", [c.D, c.G * c.HD], BF16, kind="ExternalInput")
    wot_d = nc.dram_tensor("wot", [c.HQC * c.HD, c.D], BF16, kind="ExternalInput")
    ra_d = nc.dram_tensor("ra", [c.HD, c.S], BF16, kind="ExternalInput")
    rb_d = nc.dram_tensor("rb", [c.HD, c.S], BF16, kind="ExternalInput")
    cm_d = nc.dram_tensor("cm", [128, 4 * 128], BF16, kind="ExternalInput")
    pm_d = nc.dram_tensor("pm", [128, 128], BF16, kind="ExternalInput")
    id_d = nc.dram_tensor("id", [128, 128], BF16, kind="ExternalInput")
    onec_d = nc.dram_tensor("onec", [128, 128], BF16, kind="ExternalInput")
    oner_d = nc.dram_tensor("oner", [1, 128], F32, kind="ExternalInput")
    onecf_d = nc.dram_tensor("onecf", [128, 1], F32, kind="ExternalInput")
    out_d = nc.dram_tensor("partial", [c.S, c.D], F32, kind="ExternalOutput")
    if dbg:
        qdump_d = nc.dram_tensor("qdump", [128, 16, 8, 128], BF16, kind="ExternalOutput")
        kdump_d = nc.dram_tensor("kdump", [128, 2, 16, 128], BF16, kind="ExternalOutput")
        vndump_d = nc.dram_tensor("vndump", [128, 2, 16, 128], BF16, kind="ExternalOutput")
        atdump_d = nc.dram_tensor("atdump", [128, 16, 8, 128], BF16, kind="ExternalOutput")
        dsdump_d = nc.dram_tensor("dsdump", [32, 512], F32, kind="ExternalOutput")
        zbdump_d = nc.dram_tensor("zbdump", [32, 512], F32, kind="ExternalOutput")
        otdump_d = nc.dram_tensor("otdump", [32, 512], F32, kind="ExternalOutput")

    scale = 1.0 / math.sqrt(c.HD)

    with tile.TileContext(nc) as tc, ExitStack() as ctx:
        const = ctx.enter_context(tc.tile_pool(name="const", bufs=1))
        acts = ctx.enter_context(tc.tile_pool(name="acts", bufs=1))
        xpool = ctx.enter_context(tc.tile_pool(name="xp", bufs=1))
        ptp = ctx.enter_context(tc.tile_pool(name="ptp", bufs=4))
        swpp = ctx.enter_context(tc.tile_pool(name="swpp", bufs=2))
        zbpool = ctx.enter_context(tc.tile_pool(name="zbp", bufs=2))
        rsp = ctx.enter_context(tc.tile_pool(name="rsp", bufs=2))
        dsp = ctx.enter_context(tc.tile_pool(name="dsp", bufs=2))
        orp = ctx.enter_context(tc.tile_pool(name="orp", bufs=2))

        def load_consts():
            nonlocal ra_sb, rb_sb, cm_sb, pm_sb, id_sb, ones_c, ones_r, ones_cf
            ra_sb = const.tile([128, c.QT, 128], BF16, name="ra_sb")
            nc.sync.dma_start(ra_sb[:], ra_d.rearrange("p (a q) -> p a q", q=128))
            rb_sb = const.tile([128, c.QT, 128], BF16, name="rb_sb")
            nc.sync.dma_start(rb_sb[:], rb_d.rearrange("p (a q) -> p a q", q=128))
            cm_sb = const.tile([128, 4, 128], BF16, name="cm_sb")
            nc.sync.dma_start(cm_sb[:], cm_d.rearrange("p (a q) -> p a q", q=128))
            pm_sb = const.tile([128, 128], BF16, name="pm_sb")
            nc.sync.dma_start(pm_sb[:], pm_d[:])
            id_sb = const.tile([128, 128], BF16, name="id_sb")
            nc.sync.dma_start(id_sb[:], id_d[:])
            ones_c = const.tile([128, 128], BF16, name="ones_c")
            nc.sync.dma_start(ones_c[:], onec_d[:])
            ones_r = const.tile([1, 128], F32R, name="ones_r")
            nc.sync.dma_start(ones_r[:], r(oner_d[:]))
            ones_cf = const.tile([128, 1], F32R, name="ones_cf")
            nc.sync.dma_start(ones_cf[:], r(onecf_d[:]))
        ra_sb = rb_sb = cm_sb = pm_sb = id_sb = ones_c = ones_r = ones_cf = None

        # ---- activations (SBUF-resident, bf16) ----
        q_sb = acts.tile([128, c.QT, c.HQC, 128], BF16, name="q_sb")
        kt_sb = acts.tile([128, c.G, c.QT, 128], BF16, name="kt_sb")
        vt_sb = acts.tile([128, c.G, c.QT, 128], BF16, name="vt_sb")
        vn_sb = acts.tile([128, c.G, c.QT, 128], BF16, name="vn_sb")
        at_sb = acts.tile([128, c.QT, c.HQC, 128], BF16, name="at_sb")

        def ecopy(i, dst, src):
            """Copy on engine i%2 (0=Act, 1=DVE); GPSIMD cannot read PSUM."""
            if i % 2 == 0:
                nc.scalar.copy(dst, src)
            else:
                nc.vector.tensor_copy(dst, src)

        # ================= Phase 1: QKV projections + RoPE =================
        # Two sweeps over x, one per GQA group: 6 outputs each (4 Q heads +
        # K + V), accumulated in 6 full psum banks with 512-wide moving
        # operands (4x fewer LDWEIGHTS than 128-wide tiling).
        with tc.tile_pool(name="wqkv", bufs=1) as wq_pool, tc.tile_pool(
            name="p1", bufs=1, space=bass.MemorySpace.PSUM
        ) as p1, tc.tile_pool(
            name="ps1", bufs=2, space=bass.MemorySpace.PSUM
        ) as ps:
            wq_sb = wq_pool.tile([128, c.DT, c.HQC * c.HD], BF16, name="wq_sb")
            wk_sb = wq_pool.tile([128, c.DT, c.G * c.HD], BF16, name="wk_sb")
            wv_sb = wq_pool.tile([128, c.DT, c.G * c.HD], BF16, name="wv_sb")
            wq_r = wqt_d.rearrange("(a p) h -> p a h", p=128)
            wk_r = wkt_d.rearrange("(a p) h -> p a h", p=128)
            wv_r = wvt_d.rearrange("(a p) h -> p a h", p=128)
            # dt0 slices first so the very first matmuls can start early
            nc.sync.dma_start(wq_sb[:, 0:1, :], wq_r[:, 0:1, :])
            nc.sync.dma_start(wk_sb[:, 0:1, :], wk_r[:, 0:1, :])
            nc.sync.dma_start(wv_sb[:, 0:1, :], wv_r[:, 0:1, :])

            x_r = xt_d.rearrange("(a p) s -> p a s", p=128)

            def rope_chunk(t, qsl):
                # t = t*ra + swap_pairs(t)*rb; pair swap via PE perm matmul.
                # Every write of t has a true data dep on rps so nothing can
                # clobber t before the matmul reads it (WAR-only edges proved
                # unreliable for PE moving-operand reads).
                rps = ps.tile([128, 4, 128], F32, name="rps", tag="ps")
                nc.tensor.matmul(rps[:], pm_sb[:], t)
                tmp = swpp.tile([128, 4, 128], BF16, name="tmp", tag="swp")
                nc.vector.tensor_tensor(tmp[:], t, ra_sb[:, qsl, :], MUL)
                nc.vector.tensor_tensor(t, rps[:], rb_sb[:, qsl, :], MUL)
                nc.vector.tensor_tensor(t, t, tmp[:], ADD)

            first_x_dma = [True]
            for g in range(c.G):  # sweep per GQA group
                for scc in range(4):  # 512-col chunks of S
                    qsl = slice(4 * scc, 4 * scc + 4)
                    # fresh accumulators each chunk: pool-mediated reuse
                    # enforces the drain-before-overwrite anti-dependency
                    p1t = [
                        p1.tile([128, 512], F32, name=f"p1t{i}", tag=f"p1t{i}")
                        for i in range(6)
                    ]
                    xh = [
                        xpool.tile([128, 8, 512], BF16, name=f"x{h}", tag=f"x{h}")
                        for h in range(2)
                    ]
                    if first_x_dma[0]:
                        # x dt0 slice first, then the bulk; weights stream in
                        # need-order behind it, consts last
                        nc.sync.dma_start(
                            xh[0][:, 0:1, :], x_r[:, 0:1, 0:512])
                        nc.sync.dma_start(
                            xh[0][:, 1:8, :], x_r[:, 1:8, 0:512])
                        nc.sync.dma_start(xh[1][:], x_r[:, 8:16, 0:512])
                        first_x_dma[0] = False
                        nc.sync.dma_start(wq_sb[:, 1:4, :], wq_r[:, 1:4, :])
                        nc.sync.dma_start(wk_sb[:, 1:, :], wk_r[:, 1:, :])
                        nc.sync.dma_start(wv_sb[:, 1:, :], wv_r[:, 1:, :])
                        for i in range(1, 4):
                            nc.sync.dma_start(
                                wq_sb[:, 4 * i:4 * i + 4, :], wq_r[:, 4 * i:4 * i + 4, :]
                            )
                        load_consts()
                    else:
                        for h in range(2):
                            nc.sync.dma_start(
                                xh[h][:],
                                x_r[:, 8 * h:8 * h + 8, scc * 512:(scc + 1) * 512]
                            )
                    for dt in range(c.DT):
                        xsl = xh[dt // 8][:, dt % 8, :]
                        for j in range(4):
                            nc.tensor.matmul(
                                p1t[j][:],
                                wq_sb[:, dt, (4 * g + j) * 128:(4 * g + j + 1) * 128],
                                xsl, start=(dt == 0), stop=(dt == c.DT - 1),
                            )
                        nc.tensor.matmul(
                            p1t[4][:], wk_sb[:, dt, g * 128:(g + 1) * 128],
                            xsl, start=(dt == 0), stop=(dt == c.DT - 1),
                        )
                        nc.tensor.matmul(
                            p1t[5][:], wv_sb[:, dt, g * 128:(g + 1) * 128],
                            xsl, start=(dt == 0), stop=(dt == c.DT - 1),
                        )
                    for j in range(4):
                        ecopy(j, q_sb[:, qsl, 4 * g + j, :], p1t[j][:])
                    ecopy(0, kt_sb[:, g, qsl, :], p1t[4][:])
                    ecopy(1, vt_sb[:, g, qsl, :], p1t[5][:])
                    # V^T -> V natural transposes for this chunk
                    for kt in range(4 * scc, 4 * scc + 4):
                        tp = ps.tile([128, 128], BF16, name="tp", tag="ps")
                        nc.tensor.transpose(tp[:], vt_sb[:, g, kt, :], id_sb[:])
                        ecopy(kt, vn_sb[:, g, kt, :], tp[:])
                    # RoPE on this chunk: K group g + its 4 Q heads
                    rope_chunk(kt_sb[:, g, qsl, :], qsl)
                    for j in range(4):
                        rope_chunk(q_sb[:, qsl, 4 * g + j, :], qsl)

        # ================ Phase 2+3: attention + out-projection ============
        with tc.tile_pool(name="wo", bufs=1) as wo_pool, tc.tile_pool(
            name="ps2", bufs=3, space=bass.MemorySpace.PSUM
        ) as ps, tc.tile_pool(
            name="otp", bufs=3, space=bass.MemorySpace.PSUM
        ) as otp, tc.tile_pool(
            name="o3", bufs=2, space=bass.MemorySpace.PSUM
        ) as o3p:
            wo_sb = wo_pool.tile([128, c.HQC, c.D], BF16, name="wo_sb")
            wo_r = wot_d.rearrange("(a p) d -> p a d", p=128)
            for i in range(4):
                nc.sync.dma_start(
                    wo_sb[:, 2 * i:2 * i + 2, :], wo_r[:, 2 * i:2 * i + 2, :]
                )

            # deferred tail closures: emit later to hide latency
            def attn_block(g, qt):
                """Emit scores/exp/PV/denominator for (group g, q-tile qt);
                return the deferred normalization closure."""
                nkt = qt + 1
                qmov = q_sb[:, qt, 4 * g:4 * g + 4, :]   # [128, 4, 128] moving
                ot = otp.tile([128, 512], F32, name="ot", tag="ot")
                rsum = rsp.tile([128, 512], F32R, name="rsum", tag="rsum")
                pts = {}

                def emit_S(kt):
                    stp = ps.tile([128, 512], F32, name="stp", tag="ps")
                    nc.tensor.matmul(stp[:], kt_sb[:, g, kt, :], qmov)
                    pt = ptp.tile([128, 512], BF16, name="pt", tag="pt")
                    nc.scalar.activation(pt[:], stp[:], AF.Exp, scale=scale)
                    if kt == qt:
                        nc.vector.tensor_tensor(pt[:], pt[:], cm_sb[:], MUL)
                    # partial row-sums accumulate on the DVE (off PE path)
                    if kt == 0:
                        nc.vector.tensor_copy(rsum[:], pt[:])
                    else:
                        nc.vector.tensor_tensor(rsum[:], rsum[:], pt[:], ADD)
                    pts[kt] = pt

                def emit_P(kt):
                    pt = pts.pop(kt)
                    nc.tensor.matmul(
                        ot[:], vn_sb[:, g, kt, :], pt[:],
                        start=(kt == 0), stop=(kt == nkt - 1),
                    )

                lead = min(3, nkt)
                for kt in range(lead):
                    emit_S(kt)
                for kt in range(nkt):
                    emit_P(kt)
                    if kt + lead < nkt:
                        emit_S(kt + lead)

                def tail():
                    # cross-partition sum, bcast 1/denominator, normalize
                    dps = ps.tile([1, 512], F32, name="dps", tag="ps")
                    nc.tensor.matmul(dps[:], ones_cf[:], rsum[:])
                    dsum = dsp.tile([1, 512], F32R, name="dsum", tag="dsum")
                    nc.scalar.copy(dsum[:], dps[:])
                    zbp_t = ps.tile([128, 512], F32, name="zbp", tag="ps")
                    nc.tensor.matmul(zbp_t[:], ones_r[:], dsum[:])
                    zb = zbpool.tile([128, 512], F32, name="zb", tag="zb")
                    nc.vector.reciprocal_approx_fast(zb[:], zbp_t[:])
                    nc.vector.tensor_tensor(
                        at_sb[:, qt, 4 * g:4 * g + 4, :], ot[:], zb[:], MUL,
                    )
                    if dbg:
                        nc.sync.dma_start(
                            dsdump_d[g * 16 + qt:g * 16 + qt + 1, :],
                            dsum[:].bitcast(F32))
                        nc.sync.dma_start(
                            zbdump_d[g * 16 + qt:g * 16 + qt + 1, :], zb[0:1, :])


                return tail

            def phase3_block(qt):
                orow = orp.tile([128, c.D], F32, name="orow", tag="orow")
                for dc in range(4):
                    dsl = slice(dc * 512, (dc + 1) * 512)
                    o3 = o3p.tile([128, 512], F32, name="o3", tag="o3")
                    for h in range(c.HQC):
                        nc.tensor.matmul(
                            o3[:],
                            at_sb[:, qt, h, :],
                            wo_sb[:, h, dsl],
                            start=(h == 0), stop=(h == c.HQC - 1),
                        )
                    ecopy(dc % 2, orow[:, dsl], o3[:])
                nc.sync.dma_start(out_d[qt * 128:(qt + 1) * 128, :], orow[:])

            # descending qt; phase3(qt) deferred until after attn(qt-1) so
            # the normalization chain is hidden behind PE work.
            pending_tails = []
            pending_p3 = []
            for qt in range(c.QT - 1, -1, -1):
                for g in range(c.G):
                    t = attn_block(g, qt)
                    if pending_tails:
                        pending_tails.pop(0)()
                    pending_tails.append(t)
                while pending_p3:
                    pending_p3.pop(0)()
                pending_p3.append(lambda qt=qt: phase3_block(qt))
            for t in pending_tails:
                t()
            for p in pending_p3:
                p()
            if dbg:
                nc.sync.dma_start(qdump_d[:], q_sb[:])
                nc.sync.dma_start(kdump_d[:], kt_sb[:])
                nc.sync.dma_start(vndump_d[:], vn_sb[:])
                nc.sync.dma_start(atdump_d[:], at_sb[:])

    nc.compile()
    nc.finalize()
    return nc


# ---------------------------------------------------------------------------
# Host-side sharding / gathering
# ---------------------------------------------------------------------------

def host_prep(x, freq_cis, wq, wk, wv, wo, n_cores, cfg: Cfg):
    c = cfg
    S, HD = c.S, c.HD

    x = np.asarray(x, np.float32)
    freq_cis = np.asarray(freq_cis, np.float32)
    wq = np.asarray(wq, np.float32)
    wk = np.asarray(wk, np.float32)
    wv = np.asarray(wv, np.float32)
    wo = np.asarray(wo, np.float32)

    # rope tables, interleaved layout: out[p] = ra[p]*t[p] + rb[p]*t[p^1]
    a = freq_cis[:, :, 0, 0].T
    bb = freq_cis[:, :, 0, 1].T
    cc = freq_cis[:, :, 1, 0].T
    dd = freq_cis[:, :, 1, 1].T
    ra = np.empty((HD, S), np.float32)
    rb = np.empty((HD, S), np.float32)
    ra[0::2], ra[1::2] = a, dd
    rb[0::2], rb[1::2] = bb, cc

    pm = np.zeros((HD, HD), np.float32)
    idx = np.arange(HD)
    pm[idx, idx ^ 1] = 1.0

    ks = np.arange(128)[:, None]
    qs = np.arange(128)[None, :]
    cm128 = (ks <= qs).astype(np.float32)       # [k, q] lower-tri in k<=q
    cm = np.tile(cm128, (1, 4))                  # 4 packed heads share it

    ident = np.eye(128, dtype=np.float32)

    bf = lambda arr: np.ascontiguousarray(arr).astype(bfloat16)

    in_maps = []
    for core in range(n_cores):
        b, t = core // 2, core % 2
        wq_c = wq[t * c.HQC * HD:(t + 1) * c.HQC * HD]     # [1024, D]
        wk_c = wk[t * c.G * HD:(t + 1) * c.G * HD]         # [256, D]
        wv_c = wv[t * c.G * HD:(t + 1) * c.G * HD]
        wo_c = wo[:, t * c.HQC * HD:(t + 1) * c.HQC * HD]  # [D, 1024]
        in_maps.append({
            "xt": bf(x[b].T),
            "wqt": bf(wq_c.T),
            "wkt": bf(wk_c.T),
            "wvt": bf(wv_c.T),
            "wot": bf(wo_c.T),
            "ra": bf(ra),
            "rb": bf(rb),
            "cm": bf(cm),
            "pm": bf(pm),
            "id": bf(ident),
            "onec": bf(np.ones((HD, HD), np.float32)),
            "onecf": np.ones((HD, 1), np.float32),
            "oner": np.ones((1, HD), np.float32),
        })
    return in_maps


def run(inputs: dict, n_cores: int = 8, cfg: Cfg = Cfg(), trace: bool = False,
        dbg: bool = False):
    in_maps = host_prep(
        inputs["x"], inputs["freq_cis"], inputs["wq"], inputs["wk"],
        inputs["wv"], inputs["wo"], n_cores, cfg,
    )
    nc = build_program(cfg, dbg=dbg)
    res = run_bass_kernel_spmd(nc, in_maps, list(range(n_cores)), trace=trace)
    B = cfg.B
    out = np.empty((B, cfg.S, cfg.D), np.float32)
    for b in range(B):
        out[b] = res.results[2 * b]["partial"] + res.results[2 * b + 1]["partial"]
    return out, res


def kernel(**inputs) -> np.ndarray:
    out, _ = run(inputs, n_cores=8, cfg=Cfg())
    return out


# revision 26
# speedup vs baseline: 1.1959x; 1.1959x over previous
"""Trainium2 Bass kernel for GQA attention (RoPE + causal) with output projection.

Strategy: DP(batch=4) x TP(2) over 8 NeuronCores. Core c handles batch c//2
and head-half t=c%2: 8 q-heads {8t..8t+7}, 2 kv-heads {2t, 2t+1}. Everything
is bf16 (PE runs bf16 at 1 cycle/row even for 128-wide moving operands, and
it halves DMA + SBUF), accumulation in fp32 PSUM. The score matmuls pack the
4 q-heads of a GQA group into one 512-wide moving operand against the shared
stationary K tile. Softmax row-sums accumulate on the Pool engine; the
cross-partition sum + broadcast run on the PE via ones-matmuls; the
reciprocal runs on the (broadcast) [128,512] tile so the DVE divides all
partitions in parallel. Emission is paced so the PE queue never stalls
(the hardware ramps the PE clock 1.2->2.4 GHz only under continuous load).

Host sums the 2 TP partials per batch (the all-reduce of the TP layout).
"""

import math
from contextlib import ExitStack
from dataclasses import dataclass

import numpy as np
from ml_dtypes import bfloat16

import concourse.bass as bass
import concourse.tile as tile
from concourse import bacc, mybir
from concourse.bass_utils import run_bass_kernel_spmd

F32 = mybir.dt.float32
F32R = mybir.dt.float32r
BF16 = mybir.dt.bfloat16
AF = mybir.ActivationFunctionType
MUL = mybir.AluOpType.mult
ADD = mybir.AluOpType.add


@dataclass(frozen=True)
class Cfg:
    B: int = 4       # batch (DP over 4)
    S: int = 2048    # sequence length
    D: int = 2048    # model dim
    HQC: int = 8     # q-heads per core
    G: int = 2       # kv-heads (GQA groups) per core
    HD: int = 128    # head dim

    @property
    def DT(self):
        return self.D // 128     # 16 d-tiles

    @property
    def QT(self):
        return self.S // 128     # 16 q/k tiles

    @property
    def NOUT(self):
        return self.HQC + 2 * self.G   # 12 projection outputs per core


def r(ap):
    return ap.bitcast(F32R)


def build_program(cfg: Cfg, dbg: bool = False):
    c = cfg
    nc = bacc.Bacc("TRN2", target_bir_lowering=False, debug=False)

    xt_d = nc.dram_tensor("xt", [c.D, c.S], BF16, kind="ExternalInput")
    wqt_d = nc.dram_tensor("wqt", [c.D, c.HQC * c.HD], BF16, kind="ExternalInput")
    wkt_d = nc.dram_tensor("wkt", [c.D, c.G * c.HD], BF16, kind="ExternalInput")
    wvt_d = nc.dram_tensor("wvt", [c.D, c.G * c.HD], BF16, kind="ExternalInput")
    wot_d = nc.dram_tensor("wot", [c.HQC * c.HD, c.D], BF16, kind="ExternalInput")
    ra_d = nc.dram_tensor("ra", [c.HD, c.S], BF16, kind="ExternalInput")
    rb_d = nc.dram_tensor("rb", [c.HD, c.S], BF16, kind="ExternalInput")
    cm_d = nc.dram_tensor("cm", [128, 4 * 128], BF16, kind="ExternalInput")
    pm_d = nc.dram_tensor("pm", [128, 128], BF16, kind="ExternalInput")
    id_d = nc.dram_tensor("id", [128, 128], BF16, kind="ExternalInput")
    onec_d = nc.dram_tensor("onec", [128, 128], BF16, kind="ExternalInput")
    oner_d = nc.dram_tensor("oner", [1, 128], F32, kind="ExternalInput")
    onecf_d = nc.dram_tensor("onecf", [128, 1], F32, kind="ExternalInput")
    out_d = nc.dram_tensor("partial", [c.S, c.D], F32, kind="ExternalOutput")
    if dbg:
        qdump_d = nc.dram_tensor("qdump", [128, 16, 8, 128], BF16, kind="ExternalOutput")
        kdump_d = nc.dram_tensor("kdump", [128, 2, 16, 128], BF16, kind="ExternalOutput")
        vndump_d = nc.dram_tensor("vndump", [128, 2, 16, 128], BF16, kind="ExternalOutput")
        atdump_d = nc.dram_tensor("atdump", [128, 16, 8, 128], BF16, kind="ExternalOutput")
        dsdump_d = nc.dram_tensor("dsdump", [32, 512], F32, kind="ExternalOutput")
        zbdump_d = nc.dram_tensor("zbdump", [32, 512], F32, kind="ExternalOutput")
        otdump_d = nc.dram_tensor("otdump", [32, 512], F32, kind="ExternalOutput")

    scale = 1.0 / math.sqrt(c.HD)

    with tile.TileContext(nc) as tc, ExitStack() as ctx:
        const = ctx.enter_context(tc.tile_pool(name="const", bufs=1))
        acts = ctx.enter_context(tc.tile_pool(name="acts", bufs=1))
        xpool = ctx.enter_context(tc.tile_pool(name="xp", bufs=1))
        ptp = ctx.enter_context(tc.tile_pool(name="ptp", bufs=4))
        swpp = ctx.enter_context(tc.tile_pool(name="swpp", bufs=2))
        zbpool = ctx.enter_context(tc.tile_pool(name="zbp", bufs=2))
        rsp = ctx.enter_context(tc.tile_pool(name="rsp", bufs=2))
        dsp = ctx.enter_context(tc.tile_pool(name="dsp", bufs=2))
        orp = ctx.enter_context(tc.tile_pool(name="orp", bufs=2))

        def load_consts():
            nonlocal ra_sb, rb_sb, cm_sb, pm_sb, id_sb, ones_c, ones_r, ones_cf
            ra_sb = const.tile([128, c.QT, 128], BF16, name="ra_sb")
            nc.sync.dma_start(ra_sb[:], ra_d.rearrange("p (a q) -> p a q", q=128))
            rb_sb = const.tile([128, c.QT, 128], BF16, name="rb_sb")
            nc.sync.dma_start(rb_sb[:], rb_d.rearrange("p (a q) -> p a q", q=128))
            cm_sb = const.tile([128, 4, 128], BF16, name="cm_sb")
            nc.sync.dma_start(cm_sb[:], cm_d.rearrange("p (a q) -> p a q", q=128))
            pm_sb = const.tile([128, 128], BF16, name="pm_sb")
            nc.sync.dma_start(pm_sb[:], pm_d[:])
            id_sb = const.tile([128, 128], BF16, name="id_sb")
            nc.sync.dma_start(id_sb[:], id_d[:])
            ones_c = const.tile([128, 128], BF16, name="ones_c")
            nc.sync.dma_start(ones_c[:], onec_d[:])
            ones_r = const.tile([1, 128], F32R, name="ones_r")
            nc.sync.dma_start(ones_r[:], r(oner_d[:]))
            ones_cf = const.tile([128, 1], F32R, name="ones_cf")
            nc.sync.dma_start(ones_cf[:], r(onecf_d[:]))
        ra_sb = rb_sb = cm_sb = pm_sb = id_sb = ones_c = ones_r = ones_cf = None

        # ---- activations (SBUF-resident, bf16) ----
        q_sb = acts.tile([128, c.QT, c.HQC, 128], BF16, name="q_sb")
        kt_sb = acts.tile([128, c.G, c.QT, 128], BF16, name="kt_sb")
        vt_sb = acts.tile([128, c.G, c.QT, 128], BF16, name="vt_sb")
        vn_sb = acts.tile([128, c.G, c.QT, 128], BF16, name="vn_sb")
        at_sb = acts.tile([128, c.QT, c.HQC, 128], BF16, name="at_sb")

        def ecopy(i, dst, src):
            """Copy on engine i%2 (0=Act, 1=DVE); GPSIMD cannot read PSUM."""
            if i % 2 == 0:
                nc.scalar.copy(dst, src)
            else:
                nc.vector.tensor_copy(dst, src)

        # ================= Phase 1: QKV projections + RoPE =================
        # Two sweeps over x, one per GQA group: 6 outputs each (4 Q heads +
        # K + V), accumulated in 6 full psum banks with 512-wide moving
        # operands (4x fewer LDWEIGHTS than 128-wide tiling).
        with tc.tile_pool(name="wqkv", bufs=1) as wq_pool, tc.tile_pool(
            name="p1", bufs=1, space=bass.MemorySpace.PSUM
        ) as p1, tc.tile_pool(
            name="ps1", bufs=2, space=bass.MemorySpace.PSUM
        ) as ps:
            wq_sb = wq_pool.tile([128, c.DT, c.HQC * c.HD], BF16, name="wq_sb")
            wk_sb = wq_pool.tile([128, c.DT, c.G * c.HD], BF16, name="wk_sb")
            wv_sb = wq_pool.tile([128, c.DT, c.G * c.HD], BF16, name="wv_sb")
            wq_r = wqt_d.rearrange("(a p) h -> p a h", p=128)
            wk_r = wkt_d.rearrange("(a p) h -> p a h", p=128)
            wv_r = wvt_d.rearrange("(a p) h -> p a h", p=128)
            # first-needed DMAs first: wq chunk0 / wk / wv / x, then consts
            nc.sync.dma_start(wq_sb[:, 0:4, :], wq_r[:, 0:4, :])
            nc.sync.dma_start(wk_sb[:], wk_r[:])
            nc.sync.dma_start(wv_sb[:], wv_r[:])

            x_r = xt_d.rearrange("(a p) s -> p a s", p=128)

            def rope_chunk(t, qsl):
                # t = t*ra + swap_pairs(t)*rb; pair swap via PE perm matmul.
                # Every write of t has a true data dep on rps so nothing can
                # clobber t before the matmul reads it (WAR-only edges proved
                # unreliable for PE moving-operand reads).
                rps = ps.tile([128, 4, 128], F32, name="rps", tag="ps")
                nc.tensor.matmul(rps[:], pm_sb[:], t)
                tmp = swpp.tile([128, 4, 128], BF16, name="tmp", tag="swp")
                nc.vector.tensor_tensor(tmp[:], t, ra_sb[:, qsl, :], MUL)
                nc.vector.tensor_tensor(t, rps[:], rb_sb[:, qsl, :], MUL)
                nc.vector.tensor_tensor(t, t, tmp[:], ADD)

            first_x_dma = [True]
            for g in range(c.G):  # sweep per GQA group
                for scc in range(4):  # 512-col chunks of S
                    qsl = slice(4 * scc, 4 * scc + 4)
                    # fresh accumulators each chunk: pool-mediated reuse
                    # enforces the drain-before-overwrite anti-dependency
                    p1t = [
                        p1.tile([128, 512], F32, name=f"p1t{i}", tag=f"p1t{i}")
                        for i in range(6)
                    ]
                    xh = [
                        xpool.tile([128, 8, 512], BF16, name=f"x{h}", tag=f"x{h}")
                        for h in range(2)
                    ]
                    for h in range(2):
                        nc.sync.dma_start(
                            xh[h][:],
                            x_r[:, 8 * h:8 * h + 8, scc * 512:(scc + 1) * 512]
                        )
                    if first_x_dma[0]:
                        # defer remaining weight chunks + consts behind the
                        # DMAs the first matmuls actually wait on
                        first_x_dma[0] = False
                        for i in range(1, 4):
                            nc.sync.dma_start(
                                wq_sb[:, 4 * i:4 * i + 4, :], wq_r[:, 4 * i:4 * i + 4, :]
                            )
                        load_consts()
                    for dt in range(c.DT):
                        xsl = xh[dt // 8][:, dt % 8, :]
                        for j in range(4):
                            nc.tensor.matmul(
                                p1t[j][:],
                                wq_sb[:, dt, (4 * g + j) * 128:(4 * g + j + 1) * 128],
                                xsl, start=(dt == 0), stop=(dt == c.DT - 1),
                            )
                        nc.tensor.matmul(
                            p1t[4][:], wk_sb[:, dt, g * 128:(g + 1) * 128],
                            xsl, start=(dt == 0), stop=(dt == c.DT - 1),
                        )
                        nc.tensor.matmul(
                            p1t[5][:], wv_sb[:, dt, g * 128:(g + 1) * 128],
                            xsl, start=(dt == 0), stop=(dt == c.DT - 1),
                        )
                    for j in range(4):
                        ecopy(j, q_sb[:, qsl, 4 * g + j, :], p1t[j][:])
                    ecopy(0, kt_sb[:, g, qsl, :], p1t[4][:])
                    ecopy(1, vt_sb[:, g, qsl, :], p1t[5][:])
                    # V^T -> V natural transposes for this chunk
                    for kt in range(4 * scc, 4 * scc + 4):
                        tp = ps.tile([128, 128], BF16, name="tp", tag="ps")
                        nc.tensor.transpose(tp[:], vt_sb[:, g, kt, :], id_sb[:])
                        ecopy(kt, vn_sb[:, g, kt, :], tp[:])
                    # RoPE on this chunk: K group g + its 4 Q heads
                    rope_chunk(kt_sb[:, g, qsl, :], qsl)
                    for j in range(4):
                        rope_chunk(q_sb[:, qsl, 4 * g + j, :], qsl)

        # ================ Phase 2+3: attention + out-projection ============
        with tc.tile_pool(name="wo", bufs=1) as wo_pool, tc.tile_pool(
            name="ps2", bufs=3, space=bass.MemorySpace.PSUM
        ) as ps, tc.tile_pool(
            name="otp", bufs=3, space=bass.MemorySpace.PSUM
        ) as otp, tc.tile_pool(
            name="o3", bufs=2, space=bass.MemorySpace.PSUM
        ) as o3p:
            wo_sb = wo_pool.tile([128, c.HQC, c.D], BF16, name="wo_sb")
            wo_r = wot_d.rearrange("(a p) d -> p a d", p=128)
            for i in range(4):
                nc.sync.dma_start(
                    wo_sb[:, 2 * i:2 * i + 2, :], wo_r[:, 2 * i:2 * i + 2, :]
                )

            # deferred tail closures: emit later to hide latency
            def attn_block(g, qt):
                """Emit scores/exp/PV/denominator for (group g, q-tile qt);
                return the deferred normalization closure."""
                nkt = qt + 1
                qmov = q_sb[:, qt, 4 * g:4 * g + 4, :]   # [128, 4, 128] moving
                ot = otp.tile([128, 512], F32, name="ot", tag="ot")
                rsum = rsp.tile([128, 512], F32R, name="rsum", tag="rsum")
                pts = {}

                def emit_S(kt):
                    stp = ps.tile([128, 512], F32, name="stp", tag="ps")
                    nc.tensor.matmul(stp[:], kt_sb[:, g, kt, :], qmov)
                    pt = ptp.tile([128, 512], BF16, name="pt", tag="pt")
                    nc.scalar.activation(pt[:], stp[:], AF.Exp, scale=scale)
                    if kt == qt:
                        nc.vector.tensor_tensor(pt[:], pt[:], cm_sb[:], MUL)
                    # partial row-sums accumulate on the DVE (off PE path)
                    if kt == 0:
                        nc.vector.tensor_copy(rsum[:], pt[:])
                    else:
                        nc.vector.tensor_tensor(rsum[:], rsum[:], pt[:], ADD)
                    pts[kt] = pt

                def emit_P(kt):
                    pt = pts.pop(kt)
                    nc.tensor.matmul(
                        ot[:], vn_sb[:, g, kt, :], pt[:],
                        start=(kt == 0), stop=(kt == nkt - 1),
                    )

                lead = min(3, nkt)
                for kt in range(lead):
                    emit_S(kt)
                for kt in range(nkt):
                    emit_P(kt)
                    if kt + lead < nkt:
                        emit_S(kt + lead)

                def tail():
                    # cross-partition sum, bcast 1/denominator, normalize
                    dps = ps.tile([1, 512], F32, name="dps", tag="ps")
                    nc.tensor.matmul(dps[:], ones_cf[:], rsum[:])
                    dsum = dsp.tile([1, 512], F32R, name="dsum", tag="dsum")
                    nc.scalar.copy(dsum[:], dps[:])
                    zbp_t = ps.tile([128, 512], F32, name="zbp", tag="ps")
                    nc.tensor.matmul(zbp_t[:], ones_r[:], dsum[:])
                    zb = zbpool.tile([128, 512], F32, name="zb", tag="zb")
                    nc.vector.reciprocal_approx_fast(zb[:], zbp_t[:])
                    nc.vector.tensor_tensor(
                        at_sb[:, qt, 4 * g:4 * g + 4, :], ot[:], zb[:], MUL,
                    )
                    if dbg:
                        nc.sync.dma_start(
                            dsdump_d[g * 16 + qt:g * 16 + qt + 1, :],
                            dsum[:].bitcast(F32))
                        nc.sync.dma_start(
                            zbdump_d[g * 16 + qt:g * 16 + qt + 1, :], zb[0:1, :])


                return tail

            def phase3_block(qt):
                orow = orp.tile([128, c.D], F32, name="orow", tag="orow")
                for dc in range(4):
                    dsl = slice(dc * 512, (dc + 1) * 512)
                    o3 = o3p.tile([128, 512], F32, name="o3", tag="o3")
                    for h in range(c.HQC):
                        nc.tensor.matmul(
                            o3[:],
                            at_sb[:, qt, h, :],
                            wo_sb[:, h, dsl],
                            start=(h == 0), stop=(h == c.HQC - 1),
                        )
                    ecopy(dc % 2, orow[:, dsl], o3[:])
                nc.sync.dma_start(out_d[qt * 128:(qt + 1) * 128, :], orow[:])

            # descending qt; phase3(qt) deferred until after attn(qt-1) so
            # the normalization chain is hidden behind PE work.
            pending_tails = []
            pending_p3 = []
            for qt in range(c.QT - 1, -1, -1):
                for g in range(c.G):
                    t = attn_block(g, qt)
                    if pending_tails:
                        pending_tails.pop(0)()
                    pending_tails.append(t)
                while pending_p3:
                    pending_p3.pop(0)()
                pending_p3.append(lambda qt=qt: phase3_block(qt))
            for t in pending_tails:
                t()
            for p in pending_p3:
                p()
            if dbg:
                nc.sync.dma_start(qdump_d[:], q_sb[:])
                nc.sync.dma_start(kdump_d[:], kt_sb[:])
                nc.sync.dma_start(vndump_d[:], vn_sb[:])
                nc.sync.dma_start(atdump_d[:], at_sb[:])

    nc.compile()
    nc.finalize()
    return nc


# ---------------------------------------------------------------------------
# Host-side sharding / gathering
# ---------------------------------------------------------------------------

def host_prep(x, freq_cis, wq, wk, wv, wo, n_cores, cfg: Cfg):
    c = cfg
    S, HD = c.S, c.HD

    x = np.asarray(x, np.float32)
    freq_cis = np.asarray(freq_cis, np.float32)
    wq = np.asarray(wq, np.float32)
    wk = np.asarray(wk, np.float32)
    wv = np.asarray(wv, np.float32)
    wo = np.asarray(wo, np.float32)

    # rope tables, interleaved layout: out[p] = ra[p]*t[p] + rb[p]*t[p^1]
    a = freq_cis[:, :, 0, 0].T
    bb = freq_cis[:, :, 0, 1].T
    cc = freq_cis[:, :, 1, 0].T
    dd = freq_cis[:, :, 1, 1].T
    ra = np.empty((HD, S), np.float32)
    rb = np.empty((HD, S), np.float32)
    ra[0::2], ra[1::2] = a, dd
    rb[0::2], rb[1::2] = bb, cc

    pm = np.zeros((HD, HD), np.float32)
    idx = np.arange(HD)
    pm[idx, idx ^ 1] = 1.0

    ks = np.arange(128)[:, None]
    qs = np.arange(128)[None, :]
    cm128 = (ks <= qs).astype(np.float32)       # [k, q] lower-tri in k<=q
    cm = np.tile(cm128, (1, 4))                  # 4 packed heads share it

    ident = np.eye(128, dtype=np.float32)

    bf = lambda arr: np.ascontiguousarray(arr).astype(bfloat16)

    in_maps = []
    for core in range(n_cores):
        b, t = core // 2, core % 2
        wq_c = wq[t * c.HQC * HD:(t + 1) * c.HQC * HD]     # [1024, D]
        wk_c = wk[t * c.G * HD:(t + 1) * c.G * HD]         # [256, D]
        wv_c = wv[t * c.G * HD:(t + 1) * c.G * HD]
        wo_c = wo[:, t * c.HQC * HD:(t + 1) * c.HQC * HD]  # [D, 1024]
        in_maps.append({
            "xt": bf(x[b].T),
            "wqt": bf(wq_c.T),
            "wkt": bf(wk_c.T),
            "wvt": bf(wv_c.T),
            "wot": bf(wo_c.T),
            "ra": bf(ra),
            "rb": bf(rb),
            "cm": bf(cm),
            "pm": bf(pm),
            "id": bf(ident),
            "onec": bf(np.ones((HD, HD), np.float32)),
            "onecf": np.ones((HD, 1), np.float32),
            "oner": np.ones((1, HD), np.float32),
        })
    return in_maps


def run(inputs: dict, n_cores: int = 8, cfg: Cfg = Cfg(), trace: bool = False,
        dbg: bool = False):
    in_maps = host_prep(
        inputs["x"], inputs["freq_cis"], inputs["wq"], inputs["wk"],
        inputs["wv"], inputs["wo"], n_cores, cfg,
    )
    nc = build_program(cfg, dbg=dbg)
    res = run_bass_kernel_spmd(nc, in_maps, list(range(n_cores)), trace=trace)
    B = cfg.B
    out = np.empty((B, cfg.S, cfg.D), np.float32)
    for b in range(B):
        out[b] = res.results[2 * b]["partial"] + res.results[2 * b + 1]["partial"]
    return out, res


def kernel(**inputs) -> np.ndarray:
    out, _ = run(inputs, n_cores=8, cfg=Cfg())
    return out
